# revision 20
# baseline (speedup 1.0000x reference)
"""Trainium2 Bass kernel for a cascade of 4 biquad IIR sections (DF2T).

Approach: the cascaded IIR filter is LTI with an impulse response that decays
below fp32 noise within ~32 taps (max pole modulus ~0.49 for the given
coefficient scaling).  We therefore evaluate it as an exact-to-fp32 truncated
FIR with K_TAPS=64 taps, expressed as TensorE matmuls against a pair of
128x128 Toeplitz band matrices built on the host from the (tiny) coefficient
inputs.

Layout: the input (B=512, T=32768) is transposed on the host to (T, B) so
time lies on SBUF partitions and batch on the free dim.  Output chunk q
(128 consecutive timesteps x 512 batch) is:

    y[q*128 + i, :] = sum_j h[j] * X[(q+1)*128 + i - j, :]      (X has a
                      128-row halo of history prepended)
                    = W1.T @ xtile[q] + W0.T @ xtile[q+1]

with W0[k, i] = h[i - k]        (0 <= i-k < K_TAPS)
     W1[k, i] = h[128 + i - k]  (0 <= 128+i-k < K_TAPS)

Precision/speed: MODE
  - "bf16x3" (default): x and W are split on the host into bf16 (hi, lo)
    pairs; each W.T @ x is computed as Wh@xh + Wh@xl + Wl@xh (the Wl@xl term
    is ~2^-16 relative and dropped).  bf16 matmuls run at 1 cycle/row vs 4
    for fp32, products are exact in fp32 PSUM; measured absmax error vs the
    fp64 reference is ~9e-6 of scale.  6 matmuls per output chunk.
  - "fp32": exact fp32 matmuls (4 cycles/row), absmax error ~4e-7 of scale.
    2 matmuls per output chunk, ~35% slower end-to-end.

Sharding: time is split across the 8 cores (4096 steps each + 128-row halo
from the previous shard; zeros for core 0, matching zero initial state).
Batch stays whole (512 free dim = one full PSUM bank per matmul).
"""

import os
import numpy as np
from contextlib import ExitStack

import ml_dtypes

import concourse.bass as bass
import concourse.tile as tile
from concourse import bacc, mybir
from concourse.bass_utils import run_bass_kernel_spmd

B = 512
T = 32768
NCORES = 8
T_LOC = T // NCORES            # 4096
HALO = 128
K_TAPS = 64
N_SECTIONS = 4
IN_ROWS = HALO + T_LOC         # 4224
N_IN_TILES = IN_ROWS // 128    # 33
N_CHUNKS = T_LOC // 128        # 32
IN_BATCH = 4                   # input tiles per dma_start
OUT_BATCH = 4                  # output chunks per dma_start (1 MiB)

MODE = os.environ.get("KERNEL_MODE", "bf16x3")   # "bf16x3" | "f32rx3" | "fp32"

LAST_RESULTS = None            # BassKernelResults of the most recent run
_NC_CACHE = {}


def _impulse_response(b, a, n):
    """First n taps of the cascaded DF2T biquad impulse response (float64)."""
    b = np.asarray(b, np.float64)
    a = np.asarray(a, np.float64)
    sig = np.zeros(n, np.float64)
    sig[0] = 1.0
    for k in range(N_SECTIONS):
        y = np.zeros(n, np.float64)
        s1 = 0.0
        s2 = 0.0
        for t in range(n):
            u = sig[t]
            yt = b[k, 0] * u + s1
            s1 = b[k, 1] * u - a[k, 0] * yt + s2
            s2 = b[k, 2] * u - a[k, 1] * yt
            y[t] = yt
        sig = y
    return sig


def _toeplitz_weights(b, a):
    h = _impulse_response(b, a, K_TAPS)
    k = np.arange(128)[:, None]
    i = np.arange(128)[None, :]
    j0 = i - k
    w0 = np.where((j0 >= 0) & (j0 < K_TAPS), h[np.clip(j0, 0, K_TAPS - 1)], 0.0)
    j1 = 128 + i - k
    w1 = np.where((j1 >= 0) & (j1 < K_TAPS), h[np.clip(j1, 0, K_TAPS - 1)], 0.0)
    return w0.astype(np.float32), w1.astype(np.float32)


def _split_bf16(v):
    hi = v.astype(ml_dtypes.bfloat16)
    lo = (v - hi.astype(np.float32)).astype(ml_dtypes.bfloat16)
    return hi, lo


def _split_fp16(v):
    hi = v.astype(np.float16)
    lo = (v - hi.astype(np.float32)).astype(np.float16)
    return hi, lo


def _round_mantissa(v, m=11):
    """Round fp32 to m explicit mantissa bits (the f32r operand width)."""
    u = v.view(np.uint32).astype(np.uint64)
    shift = 23 - m
    add = np.uint64(1) << np.uint64(shift - 1)
    u = ((u + add) >> np.uint64(shift)) << np.uint64(shift)
    return (u & np.uint64(0xFFFFFFFF)).astype(np.uint32).view(np.float32)


def _split_f32r(v):
    hi = _round_mantissa(v)
    lo = _round_mantissa(v - hi)
    return hi, lo


def _in_out_batches():
    # small leading input batches so the first matmuls start early, then
    # steady batches; output ramps down so the final store is small
    in_batches = []
    bi = 0
    for sz in (1, 1, 2):
        in_batches.append((bi, sz))
        bi += sz
    while bi < N_IN_TILES:
        sz = min(IN_BATCH, N_IN_TILES - bi)
        in_batches.append((bi, sz))
        bi += sz
    out_batches = []
    bo = 0
    while bo < N_CHUNKS - 4:
        out_batches.append((bo, OUT_BATCH))
        bo += OUT_BATCH
    for sz in (2, 1, 1):
        out_batches.append((bo, sz))
        bo += sz
    return in_batches, out_batches


def _build_nc(mode):
    nc = bacc.Bacc(
        "TRN2", target_bir_lowering=False, debug=False, num_devices=NCORES
    )
    f32 = mybir.dt.float32
    bf16 = mybir.dt.bfloat16
    f32r = mybir.dt.float32r
    f16 = mybir.dt.float16
    yout = nc.dram_tensor("yout", [T_LOC, B], f32, kind="ExternalOutput").ap()
    if mode == "fp32":
        xin = nc.dram_tensor("xin", [IN_ROWS, B], f32, kind="ExternalInput").ap()
        w0 = nc.dram_tensor("w0", [128, 128], f32, kind="ExternalInput").ap()
        w1 = nc.dram_tensor("w1", [128, 128], f32, kind="ExternalInput").ap()
    elif mode == "f32rx3":
        # hi/lo fp16 split of x on the wire, cast-DMA'd to f32r on chip;
        # weights are f32r (11-bit-rounded fp32) hi/lo pairs
        xin = nc.dram_tensor("xin", [2, IN_ROWS, B], f16, kind="ExternalInput").ap()
        w0 = nc.dram_tensor("w0", [2, 128, 128], f32r, kind="ExternalInput").ap()
        w1 = nc.dram_tensor("w1", [2, 128, 128], f32r, kind="ExternalInput").ap()
    else:
        # hi/lo bf16 split of x (leading dim 2) and of each weight matrix
        xin = nc.dram_tensor("xin", [2, IN_ROWS, B], bf16, kind="ExternalInput").ap()
        w0 = nc.dram_tensor("w0", [2, 128, 128], bf16, kind="ExternalInput").ap()
        w1 = nc.dram_tensor("w1", [2, 128, 128], bf16, kind="ExternalInput").ap()

    in_batches, out_batches = _in_out_batches()

    # Note: fp32 matmuls lower to a self-loading LDWEIGHTS with a single
    # sync-wait slot in walrus codegen; Bacc's compile() legalizes any
    # multi-wait instruction by hoisting extra waits into event semaphores.
    with tile.TileContext(nc) as tc, ExitStack() as ctx:
        if mode == "fp32":
            mm_dt, in_bufs_n = f32, len(in_batches)
        elif mode == "f32rx3":
            # f32r input tiles are 4B/elem: recycle 6 batches per split
            # instead of keeping all resident (SBUF budget)
            mm_dt, in_bufs_n = f32r, 12
        else:
            mm_dt, in_bufs_n = bf16, 2 * len(in_batches)

        wpool = ctx.enter_context(tc.tile_pool(name="w", bufs=4))
        warmpool = ctx.enter_context(tc.tile_pool(name="warm", bufs=2))
        inpool = ctx.enter_context(tc.tile_pool(name="xbuf", bufs=in_bufs_n))
        pspool = ctx.enter_context(tc.tile_pool(name="ps", bufs=8, space="PSUM"))
        outpool = ctx.enter_context(tc.tile_pool(name="ybuf", bufs=len(out_batches)))

        # weight tiles
        if mode == "fp32":
            w0t = wpool.tile([128, 128], f32, tag="w")
            nc.sync.dma_start(w0t[:], w0)
            w1t = wpool.tile([128, 128], f32, tag="w")
            nc.sync.dma_start(w1t[:], w1)
        else:
            wtiles = {}
            for nm, wap in (("w0", w0), ("w1", w1)):
                for s in range(2):
                    t = wpool.tile([128, 128], mm_dt, tag="w")
                    nc.sync.dma_start(t[:], wap[s])
                    wtiles[(nm, s)] = t

        # HAM warm-up: the real matmuls only start once the first input DMAs
        # land (~8us in); keep the PE busy before that with dummy bf16
        # matmuls on a memset tile so the clock gate is at 2.4 GHz (and the
        # ~3.4us warm-up window already paid) when real work begins.
        warm_in = warmpool.tile([128, 512], bf16, tag="warm_in")
        nc.gpsimd.memset(warm_in[:], 0.0)
        warm_ps = pspool.tile([128, 512], f32, tag="ps")
        for _ in range(26):
            nc.tensor.matmul(
                warm_ps[:], warm_in[:, :128], warm_in[:], start=True, stop=True
            )

        # input tiles
        tile_of = {}   # (split, j) -> AP;  fp32 mode uses split=0 only
        splits = (0,) if mode == "fp32" else (0, 1)
        for start, n in in_batches:
            for s in splits:
                t = inpool.tile([128, n * B], mm_dt, tag="xbuf")
                src = (
                    xin[start * 128 : (start + n) * 128, :]
                    if mode == "fp32"
                    else xin[s, start * 128 : (start + n) * 128, :]
                )
                # f32rx3 ships fp16 on the wire and cast-expands to f32r in
                # the DMA (SWDGE); other modes are plain HWDGE copies
                dma_eng = nc.gpsimd if mode == "f32rx3" else nc.sync
                dma_eng.dma_start(
                    t.rearrange("p (n b) -> p n b", b=B),
                    src.rearrange("(n p) b -> p n b", p=128),
                )
                for j in range(start, start + n):
                    tile_of[(s, j)] = t[:, (j - start) * B : (j - start + 1) * B]

        for bo, n in out_batches:
            ot = outpool.tile([128, n * B], f32, tag="ybuf")
            for qi in range(n):
                q = bo + qi
                pt = pspool.tile([128, B], f32, tag="ps")
                if mode == "fp32":
                    nc.tensor.matmul(
                        pt[:], w1t[:], tile_of[(0, q)], start=True, stop=False
                    )
                    nc.tensor.matmul(
                        pt[:], w0t[:], tile_of[(0, q + 1)], start=False, stop=True
                    )
                else:
                    terms = [
                        (wtiles[("w1", 0)], tile_of[(0, q)]),
                        (wtiles[("w1", 1)], tile_of[(0, q)]),
                        (wtiles[("w1", 0)], tile_of[(1, q)]),
                        (wtiles[("w0", 0)], tile_of[(0, q + 1)]),
                        (wtiles[("w0", 1)], tile_of[(0, q + 1)]),
                        (wtiles[("w0", 0)], tile_of[(1, q + 1)]),
                    ]
                    for ti, (wt, xt) in enumerate(terms):
                        nc.tensor.matmul(
                            pt[:], wt[:], xt,
                            start=(ti == 0), stop=(ti == len(terms) - 1),
                        )
                # alternate PSUM-drain between DVE and ACT so the copy stage
                # keeps up with the matmul stream on either engine; for the
                # final chunks both engines split each copy so the tail store
                # starts as early as possible
                dst = ot[:, qi * B : (qi + 1) * B]
                if q >= N_CHUNKS - 4:
                    nc.vector.tensor_copy(dst[:, : B // 2], pt[:, : B // 2])
                    nc.scalar.copy(dst[:, B // 2 :], pt[:, B // 2 :])
                elif q % 2 == 0:
                    nc.vector.tensor_copy(dst, pt[:])
                else:
                    nc.scalar.copy(dst, pt[:])
            nc.sync.dma_start(
                yout[bo * 128 : (bo + n) * 128, :].rearrange("(n p) b -> p n b", p=128),
                ot.rearrange("p (n b) -> p n b", b=B),
            )
    nc.compile()
    return nc


def _get_nc(mode):
    if mode not in _NC_CACHE:
        _NC_CACHE[mode] = _build_nc(mode)
    return _NC_CACHE[mode]


def kernel(x, b, a):
    global LAST_RESULTS
    x = np.asarray(x, np.float32)
    assert x.shape == (B, T, 1), x.shape

    xt = np.ascontiguousarray(x[:, :, 0].T)                        # (T, B)
    xpad = np.concatenate([np.zeros((HALO, B), np.float32), xt], axis=0)
    w0f, w1f = _toeplitz_weights(b, a)

    if MODE == "fp32":
        in_maps = [
            {
                "xin": np.ascontiguousarray(xpad[c * T_LOC : c * T_LOC + IN_ROWS]),
                "w0": w0f,
                "w1": w1f,
            }
            for c in range(NCORES)
        ]
    else:
        if MODE == "f32rx3":
            xh, xl = _split_fp16(xpad)
            w0s = np.stack(_split_f32r(w0f))
            w1s = np.stack(_split_f32r(w1f))
        else:
            xh, xl = _split_bf16(xpad)
            w0s = np.stack(_split_bf16(w0f))
            w1s = np.stack(_split_bf16(w1f))
        in_maps = [
            {
                "xin": np.ascontiguousarray(
                    np.stack(
                        [
                            xh[c * T_LOC : c * T_LOC + IN_ROWS],
                            xl[c * T_LOC : c * T_LOC + IN_ROWS],
                        ]
                    )
                ),
                "w0": w0s,
                "w1": w1s,
            }
            for c in range(NCORES)
        ]

    res = run_bass_kernel_spmd(_get_nc(MODE), in_maps, list(range(NCORES)))
    LAST_RESULTS = res
    yt = np.concatenate([res.results[c]["yout"] for c in range(NCORES)], axis=0)
    return np.ascontiguousarray(yt.T)[:, :, None]


# revision 21
# speedup vs baseline: 1.0105x; 1.0105x over previous
"""Trainium2 Bass kernel for a cascade of 4 biquad IIR sections (DF2T).

Approach: the cascaded IIR filter is LTI with an impulse response that decays
below fp32 noise within ~32 taps (max pole modulus ~0.49 for the given
coefficient scaling).  We therefore evaluate it as an exact-to-fp32 truncated
FIR with K_TAPS=64 taps, expressed as TensorE matmuls against a pair of
128x128 Toeplitz band matrices built on the host from the (tiny) coefficient
inputs.

Layout: the input (B=512, T=32768) is transposed on the host to (T, B) so
time lies on SBUF partitions and batch on the free dim.  Output chunk q
(128 consecutive timesteps x 512 batch) is:

    y[q*128 + i, :] = sum_j h[j] * X[(q+1)*128 + i - j, :]      (X has a
                      128-row halo of history prepended)
                    = W1.T @ xtile[q] + W0.T @ xtile[q+1]

with W0[k, i] = h[i - k]        (0 <= i-k < K_TAPS)
     W1[k, i] = h[128 + i - k]  (0 <= 128+i-k < K_TAPS)

Precision/speed: MODE
  - "bf16x3" (default): x and W are split on the host into bf16 (hi, lo)
    pairs; each W.T @ x is computed as Wh@xh + Wh@xl + Wl@xh (the Wl@xl term
    is ~2^-16 relative and dropped).  bf16 matmuls run at 1 cycle/row vs 4
    for fp32, products are exact in fp32 PSUM; measured absmax error vs the
    fp64 reference is ~9e-6 of scale.  6 matmuls per output chunk.
  - "fp32": exact fp32 matmuls (4 cycles/row), absmax error ~4e-7 of scale.
    2 matmuls per output chunk, ~35% slower end-to-end.

Sharding: time is split across the 8 cores (4096 steps each + 128-row halo
from the previous shard; zeros for core 0, matching zero initial state).
Batch stays whole (512 free dim = one full PSUM bank per matmul).
"""

import os
import numpy as np
from contextlib import ExitStack

import ml_dtypes

import concourse.bass as bass
import concourse.tile as tile
from concourse import bacc, mybir
from concourse.bass_utils import run_bass_kernel_spmd

B = 512
T = 32768
NCORES = 8
T_LOC = T // NCORES            # 4096
HALO = 128
K_TAPS = 64
N_SECTIONS = 4
IN_ROWS = HALO + T_LOC         # 4224
N_IN_TILES = IN_ROWS // 128    # 33
N_CHUNKS = T_LOC // 128        # 32
IN_BATCH = 4                   # input tiles per dma_start
OUT_BATCH = 4                  # output chunks per dma_start (1 MiB)

MODE = os.environ.get("KERNEL_MODE", "bf16x3")   # "bf16x3" | "f32rx3" | "fp32"

LAST_RESULTS = None            # BassKernelResults of the most recent run
_NC_CACHE = {}


def _impulse_response(b, a, n):
    """First n taps of the cascaded DF2T biquad impulse response (float64)."""
    b = np.asarray(b, np.float64)
    a = np.asarray(a, np.float64)
    sig = np.zeros(n, np.float64)
    sig[0] = 1.0
    for k in range(N_SECTIONS):
        y = np.zeros(n, np.float64)
        s1 = 0.0
        s2 = 0.0
        for t in range(n):
            u = sig[t]
            yt = b[k, 0] * u + s1
            s1 = b[k, 1] * u - a[k, 0] * yt + s2
            s2 = b[k, 2] * u - a[k, 1] * yt
            y[t] = yt
        sig = y
    return sig


def _toeplitz_weights(b, a):
    h = _impulse_response(b, a, K_TAPS)
    k = np.arange(128)[:, None]
    i = np.arange(128)[None, :]
    j0 = i - k
    w0 = np.where((j0 >= 0) & (j0 < K_TAPS), h[np.clip(j0, 0, K_TAPS - 1)], 0.0)
    j1 = 128 + i - k
    w1 = np.where((j1 >= 0) & (j1 < K_TAPS), h[np.clip(j1, 0, K_TAPS - 1)], 0.0)
    return w0.astype(np.float32), w1.astype(np.float32)


def _split_bf16(v):
    hi = v.astype(ml_dtypes.bfloat16)
    lo = (v - hi.astype(np.float32)).astype(ml_dtypes.bfloat16)
    return hi, lo


def _split_fp16(v):
    hi = v.astype(np.float16)
    lo = (v - hi.astype(np.float32)).astype(np.float16)
    return hi, lo


def _round_mantissa(v, m=11):
    """Round fp32 to m explicit mantissa bits (the f32r operand width)."""
    u = v.view(np.uint32).astype(np.uint64)
    shift = 23 - m
    add = np.uint64(1) << np.uint64(shift - 1)
    u = ((u + add) >> np.uint64(shift)) << np.uint64(shift)
    return (u & np.uint64(0xFFFFFFFF)).astype(np.uint32).view(np.float32)


def _split_f32r(v):
    hi = _round_mantissa(v)
    lo = _round_mantissa(v - hi)
    return hi, lo


def _in_out_batches():
    # small leading input batches so the first matmuls start early, then
    # steady batches; output ramps down so the final store is small
    in_batches = []
    bi = 0
    for sz in (1, 1, 2):
        in_batches.append((bi, sz))
        bi += sz
    while bi < N_IN_TILES:
        sz = min(IN_BATCH, N_IN_TILES - bi)
        in_batches.append((bi, sz))
        bi += sz
    out_batches = []
    bo = 0
    while bo < N_CHUNKS - 4:
        out_batches.append((bo, OUT_BATCH))
        bo += OUT_BATCH
    for sz in (2, 1, 1):
        out_batches.append((bo, sz))
        bo += sz
    return in_batches, out_batches


def _build_nc(mode):
    nc = bacc.Bacc(
        "TRN2", target_bir_lowering=False, debug=False, num_devices=NCORES
    )
    f32 = mybir.dt.float32
    bf16 = mybir.dt.bfloat16
    f32r = mybir.dt.float32r
    f16 = mybir.dt.float16
    yout = nc.dram_tensor("yout", [T_LOC, B], f32, kind="ExternalOutput").ap()
    if mode == "fp32":
        xin = nc.dram_tensor("xin", [IN_ROWS, B], f32, kind="ExternalInput").ap()
        w0 = nc.dram_tensor("w0", [128, 128], f32, kind="ExternalInput").ap()
        w1 = nc.dram_tensor("w1", [128, 128], f32, kind="ExternalInput").ap()
    elif mode == "f32rx3":
        # hi/lo fp16 split of x on the wire, cast-DMA'd to f32r on chip;
        # weights are f32r (11-bit-rounded fp32) hi/lo pairs
        xin = nc.dram_tensor("xin", [2, IN_ROWS, B], f16, kind="ExternalInput").ap()
        w0 = nc.dram_tensor("w0", [2, 128, 128], f32r, kind="ExternalInput").ap()
        w1 = nc.dram_tensor("w1", [2, 128, 128], f32r, kind="ExternalInput").ap()
    else:
        # hi/lo bf16 split of x (leading dim 2) and of each weight matrix
        xin = nc.dram_tensor("xin", [2, IN_ROWS, B], bf16, kind="ExternalInput").ap()
        w0 = nc.dram_tensor("w0", [2, 128, 128], bf16, kind="ExternalInput").ap()
        w1 = nc.dram_tensor("w1", [2, 128, 128], bf16, kind="ExternalInput").ap()

    in_batches, out_batches = _in_out_batches()

    # Note: fp32 matmuls lower to a self-loading LDWEIGHTS with a single
    # sync-wait slot in walrus codegen; Bacc's compile() legalizes any
    # multi-wait instruction by hoisting extra waits into event semaphores.
    with tile.TileContext(nc) as tc, ExitStack() as ctx:
        if mode == "fp32":
            mm_dt, in_bufs_n = f32, len(in_batches)
        elif mode == "f32rx3":
            # f32r input tiles are 4B/elem: recycle 6 batches per split
            # instead of keeping all resident (SBUF budget)
            mm_dt, in_bufs_n = f32r, 12
        else:
            mm_dt, in_bufs_n = bf16, 2 * len(in_batches)

        wpool = ctx.enter_context(tc.tile_pool(name="w", bufs=4))
        warmpool = ctx.enter_context(tc.tile_pool(name="warm", bufs=2))
        inpool = ctx.enter_context(tc.tile_pool(name="xbuf", bufs=in_bufs_n))
        pspool = ctx.enter_context(tc.tile_pool(name="ps", bufs=8, space="PSUM"))
        outpool = ctx.enter_context(tc.tile_pool(name="ybuf", bufs=len(out_batches)))

        # weight tiles
        if mode == "fp32":
            w0t = wpool.tile([128, 128], f32, tag="w")
            nc.sync.dma_start(w0t[:], w0)
            w1t = wpool.tile([128, 128], f32, tag="w")
            nc.sync.dma_start(w1t[:], w1)
        else:
            wtiles = {}
            for nm, wap in (("w0", w0), ("w1", w1)):
                for s in range(2):
                    t = wpool.tile([128, 128], mm_dt, tag="w")
                    nc.sync.dma_start(t[:], wap[s])
                    wtiles[(nm, s)] = t

        # HAM warm-up: the real matmuls only start once the first input DMAs
        # land (~8us in); keep the PE busy before that with dummy bf16
        # matmuls on a memset tile so the clock gate is at 2.4 GHz (and the
        # ~3.4us warm-up window already paid) when real work begins.
        warm_in = warmpool.tile([128, 512], bf16, tag="warm_in")
        nc.gpsimd.memset(warm_in[:], 0.0)
        warm_ps = pspool.tile([128, 512], f32, tag="ps")
        for _ in range(16):
            nc.tensor.matmul(
                warm_ps[:], warm_in[:, :128], warm_in[:], start=True, stop=True
            )

        # input tiles
        tile_of = {}   # (split, j) -> AP;  fp32 mode uses split=0 only
        splits = (0,) if mode == "fp32" else (0, 1)
        for start, n in in_batches:
            for s in splits:
                t = inpool.tile([128, n * B], mm_dt, tag="xbuf")
                src = (
                    xin[start * 128 : (start + n) * 128, :]
                    if mode == "fp32"
                    else xin[s, start * 128 : (start + n) * 128, :]
                )
                # f32rx3 ships fp16 on the wire and cast-expands to f32r in
                # the DMA (SWDGE); other modes are plain HWDGE copies
                dma_eng = nc.gpsimd if mode == "f32rx3" else nc.sync
                dma_eng.dma_start(
                    t.rearrange("p (n b) -> p n b", b=B),
                    src.rearrange("(n p) b -> p n b", p=128),
                )
                for j in range(start, start + n):
                    tile_of[(s, j)] = t[:, (j - start) * B : (j - start + 1) * B]

        for bo, n in out_batches:
            ot = outpool.tile([128, n * B], f32, tag="ybuf")
            for qi in range(n):
                q = bo + qi
                pt = pspool.tile([128, B], f32, tag="ps")
                if mode == "fp32":
                    nc.tensor.matmul(
                        pt[:], w1t[:], tile_of[(0, q)], start=True, stop=False
                    )
                    nc.tensor.matmul(
                        pt[:], w0t[:], tile_of[(0, q + 1)], start=False, stop=True
                    )
                else:
                    terms = [
                        (wtiles[("w1", 0)], tile_of[(0, q)]),
                        (wtiles[("w1", 1)], tile_of[(0, q)]),
                        (wtiles[("w1", 0)], tile_of[(1, q)]),
                        (wtiles[("w0", 0)], tile_of[(0, q + 1)]),
                        (wtiles[("w0", 1)], tile_of[(0, q + 1)]),
                        (wtiles[("w0", 0)], tile_of[(1, q + 1)]),
                    ]
                    for ti, (wt, xt) in enumerate(terms):
                        nc.tensor.matmul(
                            pt[:], wt[:], xt,
                            start=(ti == 0), stop=(ti == len(terms) - 1),
                        )
                # alternate PSUM-drain between DVE and ACT so the copy stage
                # keeps up with the matmul stream on either engine; for the
                # final chunks both engines split each copy so the tail store
                # starts as early as possible
                dst = ot[:, qi * B : (qi + 1) * B]
                if q >= N_CHUNKS - 4:
                    nc.vector.tensor_copy(dst[:, : B // 2], pt[:, : B // 2])
                    nc.scalar.copy(dst[:, B // 2 :], pt[:, B // 2 :])
                elif q % 2 == 0:
                    nc.vector.tensor_copy(dst, pt[:])
                else:
                    nc.scalar.copy(dst, pt[:])
            nc.sync.dma_start(
                yout[bo * 128 : (bo + n) * 128, :].rearrange("(n p) b -> p n b", p=128),
                ot.rearrange("p (n b) -> p n b", b=B),
            )
    nc.compile()
    return nc


def _get_nc(mode):
    if mode not in _NC_CACHE:
        _NC_CACHE[mode] = _build_nc(mode)
    return _NC_CACHE[mode]


def kernel(x, b, a):
    global LAST_RESULTS
    x = np.asarray(x, np.float32)
    assert x.shape == (B, T, 1), x.shape

    xt = np.ascontiguousarray(x[:, :, 0].T)                        # (T, B)
    xpad = np.concatenate([np.zeros((HALO, B), np.float32), xt], axis=0)
    w0f, w1f = _toeplitz_weights(b, a)

    if MODE == "fp32":
        in_maps = [
            {
                "xin": np.ascontiguousarray(xpad[c * T_LOC : c * T_LOC + IN_ROWS]),
                "w0": w0f,
                "w1": w1f,
            }
            for c in range(NCORES)
        ]
    else:
        if MODE == "f32rx3":
            xh, xl = _split_fp16(xpad)
            w0s = np.stack(_split_f32r(w0f))
            w1s = np.stack(_split_f32r(w1f))
        else:
            xh, xl = _split_bf16(xpad)
            w0s = np.stack(_split_bf16(w0f))
            w1s = np.stack(_split_bf16(w1f))
        in_maps = [
            {
                "xin": np.ascontiguousarray(
                    np.stack(
                        [
                            xh[c * T_LOC : c * T_LOC + IN_ROWS],
                            xl[c * T_LOC : c * T_LOC + IN_ROWS],
                        ]
                    )
                ),
                "w0": w0s,
                "w1": w1s,
            }
            for c in range(NCORES)
        ]

    res = run_bass_kernel_spmd(_get_nc(MODE), in_maps, list(range(NCORES)))
    LAST_RESULTS = res
    yt = np.concatenate([res.results[c]["yout"] for c in range(NCORES)], axis=0)
    return np.ascontiguousarray(yt.T)[:, :, None]
